# revision 22
# baseline (speedup 1.0000x reference)
"""Trainium2 Bass kernel for nn_BatchCropElements: out = x * (rand_u > 0.3).

Full inputs: x [64, 2048, 24, 12] f32, rand_u [24, 12] f32.
Sharding: data-parallel on batch across 8 cores -> per-core x [8, 2048, 24, 12],
viewed flat as [128 partitions, 36864 free] f32 (36864 = 128 spatial planes of
288 = 24*12 elements, so the mask pattern tiles the free dim exactly).
rand_u is replicated to every core (host pre-broadcasts it to [128, 288]).

Per core: build the f32 0/1 mask once in SBUF (threshold + log-doubling widen
to one chunk width), then stream 8 chunks of [128, 4608]: HWDGE load (sync) ->
DVE tensor-tensor multiply in place -> one SWDGE store (gpsimd) per chunk
pair. Memory-bound: measured ~102-108 us/core on HW = ~360 GB/s/core combined
read+write, i.e. at the per-NeuronCore HBM roofline.
"""

from contextlib import ExitStack

import numpy as np

import concourse.bass as bass
import concourse.tile as tile
from concourse import bacc, mybir
from concourse.bass_utils import run_bass_kernel_spmd

N_CORES = 8
B, C, H, W = 64, 2048, 24, 12
HW = H * W  # 288
B_SH = B // N_CORES  # 8 batches per core
P = 128
F_TOTAL = B_SH * C * HW // P  # 36864 f32 per partition
F = 4608  # chunk free size (16 spatial planes); F % HW == 0
N_CHUNK = F_TOTAL // F  # 8
PROB = 0.3

_DT = mybir.dt.float32


def _build_nc() -> bass.Bass:
    # Bacc (not raw Bass): its finalize pipeline splits multi-wait sync into
    # event-semaphore chains — TRN2 allows at most 1 wait per instruction.
    nc = bacc.Bacc()
    x = nc.declare_dram_parameter("x", [P, F_TOTAL], _DT, isOutput=False)
    u = nc.declare_dram_parameter("u", [P, HW], _DT, isOutput=False)
    out = nc.declare_dram_parameter("out", [P, F_TOTAL], _DT, isOutput=True)

    # The walrus TensorTensor encoding fits only one embedded sync wait, so
    # the structure keeps every DVE op at <=1 wait: bufs == N_CHUNK (no SBUF
    # slot reuse -> loads need no WAR wait, muls wait only on their own
    # load), in-place multiply, and a 1-element "absorber" copy that soaks up
    # the mask-ready wait so the first mul doesn't carry two.
    with tile.TileContext(nc) as tc:
        with (
            tc.tile_pool(name="upool", bufs=1) as upool,
            tc.tile_pool(name="maskp", bufs=1) as maskp,
            tc.tile_pool(name="scrp", bufs=1) as scrp,
            tc.tile_pool(name="iop", bufs=N_CHUNK // 2) as iop,
        ):
            tu = upool.tile([P, HW], _DT)
            nc.sync.dma_start(out=tu[:], in_=u[:, :])
            bmask = maskp.tile([P, F], _DT)
            nc.vector.tensor_scalar(
                out=bmask[:, 0:HW],
                in0=tu[:],
                scalar1=PROB,
                scalar2=None,
                op0=mybir.AluOpType.is_gt,
            )
            w = HW
            while w < F:
                nc.vector.tensor_copy(out=bmask[:, w : 2 * w], in_=bmask[:, 0:w])
                w *= 2
            scr = scrp.tile([1, 1], _DT)
            nc.vector.tensor_copy(out=scr[:], in_=bmask[0:1, F - 1 : F])

            # Paired chunks: 8 loads on the HWDGE lanes, but one SWDGE store
            # per [P, 2F] double-tile so only 4 DMA-SW lanes are used. Keeps
            # the kernel-tail drain's sem-wait list (1 DVE + 8 HW + 4 SW)
            # under the CTRL struct's capacity, stores at 1 wait (DVE), and
            # stores off the HWDGE lanes where reuse would add a second wait.
            for k in range(N_CHUNK // 2):
                t = iop.tile([P, 2 * F], _DT, name="t")
                for h in range(2):
                    c = 2 * k + h
                    sl = slice(h * F, (h + 1) * F)
                    nc.sync.dma_start(out=t[:, sl], in_=x[:, c * F : (c + 1) * F])
                    nc.vector.tensor_mul(out=t[:, sl], in0=t[:, sl], in1=bmask[:])
                nc.gpsimd.dma_start(
                    out=out[:, 2 * k * F : 2 * (k + 1) * F], in_=t[:]
                )
    nc.finalize()
    return nc


def _build_nc_raw() -> bass.Bass:
    """Raw Bacc variant: hand-rolled semaphores, no Tile tail barrier.

    SP issues the mask load + 8 chunk loads (HWDGE), each with its own
    semaphore (a shared DMA sem cannot order per-chunk completion: the 16
    SDMA engine slots of different in-flight DMAs interleave increments).
    DVE builds the mask then multiplies each chunk in place; GpSimd issues
    one SWDGE store per chunk pair and ends with a single wait for all
    store completions.
    """
    nc = bacc.Bacc()
    x = nc.declare_dram_parameter("x", [P, F_TOTAL], _DT, isOutput=False)
    u = nc.declare_dram_parameter("u", [P, HW], _DT, isOutput=False)
    out = nc.declare_dram_parameter("out", [P, F_TOTAL], _DT, isOutput=True)

    with ExitStack() as ctx:
        tu = ctx.enter_context(nc.sbuf_tensor("tu", [P, HW], _DT))
        bmask = ctx.enter_context(nc.sbuf_tensor("bmask", [P, F], _DT))
        # Double-width buffers: halves are loaded/multiplied per chunk, the
        # whole buffer is stored in one SWDGE DMA per pair.
        pairs = [
            ctx.enter_context(nc.sbuf_tensor(f"pair{k}", [P, 2 * F], _DT))
            for k in range(N_CHUNK // 2)
        ]
        ts = [pairs[c // 2][:, (c % 2) * F : (c % 2 + 1) * F] for c in range(N_CHUNK)]
        ld_sems = [
            ctx.enter_context(nc.semaphore(f"ld{c}")) for c in range(N_CHUNK + 1)
        ]
        mul_sem = ctx.enter_context(nc.semaphore("mul"))
        mk_sem = ctx.enter_context(nc.semaphore("mk"))
        st_sem = ctx.enter_context(nc.semaphore("st"))
        block = ctx.enter_context(nc.Block())

        @block.sync
        def _(sync):
            sync.dma_start(out=tu[:], in_=u[:, :]).then_inc(ld_sems[0], 16)
            for c in range(N_CHUNK):
                sync.dma_start(
                    out=ts[c], in_=x[:, c * F : (c + 1) * F]
                ).then_inc(ld_sems[c + 1], 16)

        @block.vector
        def _(vector):
            vector.wait_ge(ld_sems[0], 16)
            # DVE is pipelined: same-engine RAW chains need explicit sems.
            vector.tensor_scalar(
                out=bmask[:, 0:HW],
                in0=tu[:],
                scalar1=PROB,
                scalar2=None,
                op0=mybir.AluOpType.is_gt,
            ).then_inc(mk_sem, 1)
            w = HW
            n_mk = 1
            while w < F:
                vector.wait_ge(mk_sem, n_mk)
                vector.tensor_copy(
                    out=bmask[:, w : 2 * w], in_=bmask[:, 0:w]
                ).then_inc(mk_sem, 1)
                w *= 2
                n_mk += 1
            for c in range(N_CHUNK):
                if c == 0:
                    vector.wait_ge(mk_sem, n_mk)
                vector.wait_ge(ld_sems[c + 1], 16)
                vector.tensor_tensor(
                    out=ts[c],
                    in0=ts[c],
                    in1=bmask[:],
                    op=mybir.AluOpType.mult,
                ).then_inc(mul_sem, 1)

        @block.gpsimd
        def _(gpsimd):
            for k in range(N_CHUNK // 2):
                gpsimd.wait_ge(mul_sem, 2 * k + 2)
                gpsimd.dma_start(
                    out=out[:, 2 * k * F : 2 * (k + 1) * F],
                    in_=pairs[k][:],
                ).then_inc(st_sem, 16)
            gpsimd.wait_ge(st_sem, 16 * (N_CHUNK // 2))

    nc.finalize()
    return nc


def _build_nc_taper() -> bass.Bass:
    """Tile variant with tapered chunk sizes: big chunks stream first, tiny
    chunks last, so the serial endgame (last load -> last mul -> last store)
    is a few hundred KB instead of 4.5 MB."""
    nc = bacc.Bacc()
    x = nc.declare_dram_parameter("x", [P, F_TOTAL], _DT, isOutput=False)
    u = nc.declare_dram_parameter("u", [P, HW], _DT, isOutput=False)
    out = nc.declare_dram_parameter("out", [P, F_TOTAL], _DT, isOutput=True)

    # chunk sizes in spatial planes (x HW f32 along the free dim); sums to 128
    plan = [16, 16, 16, 16, 16, 16, 16, 8, 4, 2, 1, 1]
    assert sum(plan) == F_TOTAL // HW
    # store groups: consecutive chunk-index pairs share one SBUF buffer
    groups = [(0, 1), (2, 3), (4, 5), (6, 7), (8, 9), (10, 11)]

    with tile.TileContext(nc) as tc:
        with (
            tc.tile_pool(name="upool", bufs=1) as upool,
            tc.tile_pool(name="maskp", bufs=1) as maskp,
            tc.tile_pool(name="iop", bufs=1) as iop,
        ):
            tu = upool.tile([P, HW], _DT)
            nc.sync.dma_start(out=tu[:], in_=u[:, :])
            bmask = maskp.tile([P, F], _DT)
            nc.vector.tensor_scalar(
                out=bmask[:, 0:HW],
                in0=tu[:],
                scalar1=PROB,
                scalar2=None,
                op0=mybir.AluOpType.is_gt,
            )
            w = HW
            while w < F:
                nc.vector.tensor_copy(out=bmask[:, w : 2 * w], in_=bmask[:, 0:w])
                w *= 2

            starts = [sum(plan[:i]) * HW for i in range(len(plan) + 1)]
            for gi, grp in enumerate(groups):
                g_lo, g_hi = starts[grp[0]], starts[grp[-1] + 1]
                t = iop.tile([P, g_hi - g_lo], _DT, name=f"t{gi}")
                for c in grp:
                    lo, hi = starts[c] - g_lo, starts[c + 1] - g_lo
                    nc.sync.dma_start(
                        out=t[:, lo:hi], in_=x[:, starts[c] : starts[c + 1]]
                    )
                    fw = hi - lo
                    nc.vector.tensor_mul(
                        out=t[:, lo:hi], in0=t[:, lo:hi], in1=bmask[:, 0:fw]
                    )
                nc.gpsimd.dma_start(out=out[:, g_lo:g_hi], in_=t[:])
    nc.finalize()
    return nc


BUILDER = "tile"
_NC_CACHE: dict = {}


def _run(inputs: dict, trace: bool = False):
    x = np.ascontiguousarray(inputs["x"], dtype=np.float32)
    rand_u = np.ascontiguousarray(inputs["rand_u"], dtype=np.float32)
    assert x.shape == (B, C, H, W), x.shape
    assert rand_u.shape == (H, W), rand_u.shape

    u_rep = np.ascontiguousarray(
        np.broadcast_to(rand_u.reshape(1, HW), (P, HW)), dtype=np.float32
    )
    in_maps = []
    for i in range(N_CORES):
        shard = x[i * B_SH : (i + 1) * B_SH].reshape(P, F_TOTAL)
        in_maps.append({"x": shard, "u": u_rep})

    key = BUILDER
    if key not in _NC_CACHE:
        _NC_CACHE[key] = {
            "tile": _build_nc,
            "raw": _build_nc_raw,
            "taper": _build_nc_taper,
        }[key]()
    nc = _NC_CACHE[key]

    res = run_bass_kernel_spmd(nc, in_maps, list(range(N_CORES)), trace=trace)
    out = np.empty((B, C, H, W), dtype=np.float32)
    for i in range(N_CORES):
        out[i * B_SH : (i + 1) * B_SH] = res.results[i]["out"].reshape(
            B_SH, C, H, W
        )
    return out, res


def kernel(**inputs: np.ndarray) -> np.ndarray:
    out, _ = _run(inputs, trace=False)
    return out


# revision 28
# speedup vs baseline: 1.0347x; 1.0347x over previous
"""Trainium2 Bass kernel for nn_BatchCropElements: out = x * (rand_u > 0.3).

Full inputs: x [64, 2048, 24, 12] f32, rand_u [24, 12] f32.
Sharding: data-parallel on batch across 8 cores -> per-core x [8, 2048, 24, 12],
viewed flat as [128 partitions, 36864 free] f32 (36864 = 128 spatial planes of
288 = 24*12 elements, so the mask pattern tiles the free dim exactly).
rand_u is replicated to every core (host pre-broadcasts it to [128, 288]).

Per core: build the f32 0/1 mask once in SBUF (threshold + log-doubling
widen to chunk width), then stream 8 chunks of [128, 4608]: HWDGE load on
the SP ring -> DVE tensor-tensor multiply in place -> HWDGE store on the ACT
ring (no GpSimd: avoids its library preamble and SWDGE descriptor costs; the
two rings round-robin packets so reads and writes share the fabric fairly).
Memory-bound: the DMA fabric sustains ~433 GB/s/core combined read+write;
measured best ~100.8 us/core on HW (~9 us fixed NEFF startup + 37.9 MB of
HBM traffic + tail barrier), with device-state noise up to ~120 us.
"""

from contextlib import ExitStack

import numpy as np

import concourse.bass as bass
import concourse.tile as tile
from concourse import bacc, mybir
from concourse.bass_utils import run_bass_kernel_spmd

N_CORES = 8
B, C, H, W = 64, 2048, 24, 12
HW = H * W  # 288
B_SH = B // N_CORES  # 8 batches per core
P = 128
F_TOTAL = B_SH * C * HW // P  # 36864 f32 per partition
F = 4608  # chunk free size (16 spatial planes); F % HW == 0
N_CHUNK = F_TOTAL // F  # 8
PROB = 0.3

_DT = mybir.dt.float32


def _build_nc() -> bass.Bass:
    # Bacc (not raw Bass): its finalize pipeline splits multi-wait sync into
    # event-semaphore chains — TRN2 allows at most 1 wait per instruction.
    nc = bacc.Bacc()
    x = nc.declare_dram_parameter("x", [P, F_TOTAL], _DT, isOutput=False)
    u = nc.declare_dram_parameter("u", [P, HW], _DT, isOutput=False)
    out = nc.declare_dram_parameter("out", [P, F_TOTAL], _DT, isOutput=True)

    # The walrus TensorTensor encoding fits only one embedded sync wait, so
    # the structure keeps every DVE op at <=1 wait: bufs == N_CHUNK (no SBUF
    # slot reuse -> loads need no WAR wait, muls wait only on their own
    # load), in-place multiply, and a 1-element "absorber" copy that soaks up
    # the mask-ready wait so the first mul doesn't carry two.
    with tile.TileContext(nc) as tc:
        with (
            tc.tile_pool(name="upool", bufs=1) as upool,
            tc.tile_pool(name="maskp", bufs=1) as maskp,
            tc.tile_pool(name="scrp", bufs=1) as scrp,
            tc.tile_pool(name="iop", bufs=N_CHUNK // 2) as iop,
        ):
            tu = upool.tile([P, HW], _DT)
            nc.sync.dma_start(out=tu[:], in_=u[:, :])
            bmask = maskp.tile([P, F], _DT)
            nc.vector.tensor_scalar(
                out=bmask[:, 0:HW],
                in0=tu[:],
                scalar1=PROB,
                scalar2=None,
                op0=mybir.AluOpType.is_gt,
            )
            w = HW
            while w < F:
                nc.vector.tensor_copy(out=bmask[:, w : 2 * w], in_=bmask[:, 0:w])
                w *= 2
            scr = scrp.tile([1, 1], _DT)
            nc.vector.tensor_copy(out=scr[:], in_=bmask[0:1, F - 1 : F])

            # Paired chunks: 8 loads on the HWDGE lanes, but one SWDGE store
            # per [P, 2F] double-tile so only 4 DMA-SW lanes are used. Keeps
            # the kernel-tail drain's sem-wait list (1 DVE + 8 HW + 4 SW)
            # under the CTRL struct's capacity, stores at 1 wait (DVE), and
            # stores off the HWDGE lanes where reuse would add a second wait.
            for k in range(N_CHUNK // 2):
                t = iop.tile([P, 2 * F], _DT, name="t")
                for h in range(2):
                    c = 2 * k + h
                    sl = slice(h * F, (h + 1) * F)
                    nc.sync.dma_start(out=t[:, sl], in_=x[:, c * F : (c + 1) * F])
                    nc.vector.tensor_mul(out=t[:, sl], in0=t[:, sl], in1=bmask[:])
                nc.gpsimd.dma_start(
                    out=out[:, 2 * k * F : 2 * (k + 1) * F], in_=t[:]
                )
    nc.finalize()
    return nc


def _build_nc_raw() -> bass.Bass:
    """Raw Bacc variant: hand-rolled semaphores, no Tile tail barrier.

    SP issues the mask load + 8 chunk loads (HWDGE), each with its own
    semaphore (a shared DMA sem cannot order per-chunk completion: the 16
    SDMA engine slots of different in-flight DMAs interleave increments).
    DVE builds the mask then multiplies each chunk in place; GpSimd issues
    one SWDGE store per chunk pair and ends with a single wait for all
    store completions.
    """
    nc = bacc.Bacc()
    x = nc.declare_dram_parameter("x", [P, F_TOTAL], _DT, isOutput=False)
    u = nc.declare_dram_parameter("u", [P, HW], _DT, isOutput=False)
    out = nc.declare_dram_parameter("out", [P, F_TOTAL], _DT, isOutput=True)

    with ExitStack() as ctx:
        tu = ctx.enter_context(nc.sbuf_tensor("tu", [P, HW], _DT))
        bmask = ctx.enter_context(nc.sbuf_tensor("bmask", [P, F], _DT))
        # Double-width buffers: halves are loaded/multiplied per chunk, the
        # whole buffer is stored in one SWDGE DMA per pair.
        pairs = [
            ctx.enter_context(nc.sbuf_tensor(f"pair{k}", [P, 2 * F], _DT))
            for k in range(N_CHUNK // 2)
        ]
        ts = [pairs[c // 2][:, (c % 2) * F : (c % 2 + 1) * F] for c in range(N_CHUNK)]
        ld_sems = [
            ctx.enter_context(nc.semaphore(f"ld{c}")) for c in range(N_CHUNK + 1)
        ]
        mul_sem = ctx.enter_context(nc.semaphore("mul"))
        mk_sem = ctx.enter_context(nc.semaphore("mk"))
        st_sem = ctx.enter_context(nc.semaphore("st"))
        block = ctx.enter_context(nc.Block())

        @block.sync
        def _(sync):
            sync.dma_start(out=tu[:], in_=u[:, :]).then_inc(ld_sems[0], 16)
            for c in range(N_CHUNK):
                sync.dma_start(
                    out=ts[c], in_=x[:, c * F : (c + 1) * F]
                ).then_inc(ld_sems[c + 1], 16)

        @block.vector
        def _(vector):
            vector.wait_ge(ld_sems[0], 16)
            # DVE is pipelined: same-engine RAW chains need explicit sems.
            vector.tensor_scalar(
                out=bmask[:, 0:HW],
                in0=tu[:],
                scalar1=PROB,
                scalar2=None,
                op0=mybir.AluOpType.is_gt,
            ).then_inc(mk_sem, 1)
            w = HW
            n_mk = 1
            while w < F:
                vector.wait_ge(mk_sem, n_mk)
                vector.tensor_copy(
                    out=bmask[:, w : 2 * w], in_=bmask[:, 0:w]
                ).then_inc(mk_sem, 1)
                w *= 2
                n_mk += 1
            for c in range(N_CHUNK):
                if c == 0:
                    vector.wait_ge(mk_sem, n_mk)
                vector.wait_ge(ld_sems[c + 1], 16)
                vector.tensor_tensor(
                    out=ts[c],
                    in0=ts[c],
                    in1=bmask[:],
                    op=mybir.AluOpType.mult,
                ).then_inc(mul_sem, 1)

        @block.gpsimd
        def _(gpsimd):
            for k in range(N_CHUNK // 2):
                gpsimd.wait_ge(mul_sem, 2 * k + 2)
                gpsimd.dma_start(
                    out=out[:, 2 * k * F : 2 * (k + 1) * F],
                    in_=pairs[k][:],
                ).then_inc(st_sem, 16)
            gpsimd.wait_ge(st_sem, 16 * (N_CHUNK // 2))

    nc.finalize()
    return nc


def _build_nc_taper() -> bass.Bass:
    """Tile variant with tapered chunk sizes: big chunks stream first, tiny
    chunks last, so the serial endgame (last load -> last mul -> last store)
    is a few hundred KB instead of 4.5 MB."""
    nc = bacc.Bacc()
    x = nc.declare_dram_parameter("x", [P, F_TOTAL], _DT, isOutput=False)
    u = nc.declare_dram_parameter("u", [P, HW], _DT, isOutput=False)
    out = nc.declare_dram_parameter("out", [P, F_TOTAL], _DT, isOutput=True)

    # chunk sizes in spatial planes (x HW f32 along the free dim); sums to 128
    plan = [16, 16, 16, 16, 16, 16, 16, 8, 4, 2, 1, 1]
    assert sum(plan) == F_TOTAL // HW
    # store groups: consecutive chunk-index pairs share one SBUF buffer
    groups = [(0, 1), (2, 3), (4, 5), (6, 7), (8, 9), (10, 11)]

    with tile.TileContext(nc) as tc:
        with (
            tc.tile_pool(name="upool", bufs=1) as upool,
            tc.tile_pool(name="maskp", bufs=1) as maskp,
            tc.tile_pool(name="iop", bufs=1) as iop,
        ):
            tu = upool.tile([P, HW], _DT)
            nc.sync.dma_start(out=tu[:], in_=u[:, :])
            bmask = maskp.tile([P, F], _DT)
            nc.vector.tensor_scalar(
                out=bmask[:, 0:HW],
                in0=tu[:],
                scalar1=PROB,
                scalar2=None,
                op0=mybir.AluOpType.is_gt,
            )
            w = HW
            while w < F:
                nc.vector.tensor_copy(out=bmask[:, w : 2 * w], in_=bmask[:, 0:w])
                w *= 2

            starts = [sum(plan[:i]) * HW for i in range(len(plan) + 1)]
            for gi, grp in enumerate(groups):
                g_lo, g_hi = starts[grp[0]], starts[grp[-1] + 1]
                t = iop.tile([P, g_hi - g_lo], _DT, name=f"t{gi}")
                for c in grp:
                    lo, hi = starts[c] - g_lo, starts[c + 1] - g_lo
                    nc.sync.dma_start(
                        out=t[:, lo:hi], in_=x[:, starts[c] : starts[c + 1]]
                    )
                    fw = hi - lo
                    nc.vector.tensor_mul(
                        out=t[:, lo:hi], in0=t[:, lo:hi], in1=bmask[:, 0:fw]
                    )
                nc.gpsimd.dma_start(out=out[:, g_lo:g_hi], in_=t[:])
    nc.finalize()
    return nc


def _build_nc_hw(n_chunk: int) -> bass.Bass:
    """All-HWDGE variant: loads on the SP ring, stores on the ACT ring, no
    GpSimd at all (drops its library preamble and the ~1us/DMA SWDGE
    descriptor cost). One store per chunk for fine R/W interleave; Bacc's
    finalize legalizes the lane-reuse multi-waits this creates."""
    f = F_TOTAL // n_chunk
    assert f % HW == 0
    nc = bacc.Bacc()
    x = nc.declare_dram_parameter("x", [P, F_TOTAL], _DT, isOutput=False)
    u = nc.declare_dram_parameter("u", [P, HW], _DT, isOutput=False)
    out = nc.declare_dram_parameter("out", [P, F_TOTAL], _DT, isOutput=True)

    with tile.TileContext(nc) as tc:
        with (
            tc.tile_pool(name="upool", bufs=1) as upool,
            tc.tile_pool(name="maskp", bufs=1) as maskp,
            tc.tile_pool(name="iop", bufs=n_chunk) as iop,
        ):
            tu = upool.tile([P, HW], _DT)
            nc.scalar.dma_start(out=tu[:], in_=u[:, :])
            bmask = maskp.tile([P, f], _DT)
            nc.vector.tensor_scalar(
                out=bmask[:, 0:HW],
                in0=tu[:],
                scalar1=PROB,
                scalar2=None,
                op0=mybir.AluOpType.is_gt,
            )
            w = HW
            while w < f:
                nc.vector.tensor_copy(out=bmask[:, w : 2 * w], in_=bmask[:, 0:w])
                w *= 2
            for c in range(n_chunk):
                t = iop.tile([P, f], _DT, name="t")
                nc.sync.dma_start(out=t[:], in_=x[:, c * f : (c + 1) * f])
                nc.vector.tensor_mul(out=t[:], in0=t[:], in1=bmask[:])
                nc.scalar.dma_start(out=out[:, c * f : (c + 1) * f], in_=t[:])
    nc.finalize()
    return nc


def _build_nc_hwtaper() -> bass.Bass:
    """hw8's all-HWDGE two-ring structure plus an end-tapered chunk plan so
    the serial endgame (last load -> last mul -> last store) shrinks from
    ~2.25 MB to ~150 KB."""
    plan = [16, 16, 16, 16, 16, 16, 16, 8, 4, 2, 1, 1]
    assert sum(plan) == F_TOTAL // HW
    fmax = max(plan) * HW
    nc = bacc.Bacc()
    x = nc.declare_dram_parameter("x", [P, F_TOTAL], _DT, isOutput=False)
    u = nc.declare_dram_parameter("u", [P, HW], _DT, isOutput=False)
    out = nc.declare_dram_parameter("out", [P, F_TOTAL], _DT, isOutput=True)

    with tile.TileContext(nc) as tc:
        with (
            tc.tile_pool(name="upool", bufs=1) as upool,
            tc.tile_pool(name="maskp", bufs=1) as maskp,
            tc.tile_pool(name="iop", bufs=1) as iop,
        ):
            tu = upool.tile([P, HW], _DT)
            nc.scalar.dma_start(out=tu[:], in_=u[:, :])
            bmask = maskp.tile([P, fmax], _DT)
            nc.vector.tensor_scalar(
                out=bmask[:, 0:HW],
                in0=tu[:],
                scalar1=PROB,
                scalar2=None,
                op0=mybir.AluOpType.is_gt,
            )
            w = HW
            while w < fmax:
                nc.vector.tensor_copy(out=bmask[:, w : 2 * w], in_=bmask[:, 0:w])
                w *= 2
            starts = [sum(plan[:i]) * HW for i in range(len(plan) + 1)]
            for c in range(len(plan)):
                fw = starts[c + 1] - starts[c]
                t = iop.tile([P, fw], _DT, name=f"t{c}")
                nc.sync.dma_start(out=t[:], in_=x[:, starts[c] : starts[c + 1]])
                nc.vector.tensor_mul(out=t[:], in0=t[:], in1=bmask[:, 0:fw])
                nc.scalar.dma_start(out=out[:, starts[c] : starts[c + 1]], in_=t[:])
    nc.finalize()
    return nc


BUILDER = "hw8"
_NC_CACHE: dict = {}


def _run(inputs: dict, trace: bool = False):
    x = np.ascontiguousarray(inputs["x"], dtype=np.float32)
    rand_u = np.ascontiguousarray(inputs["rand_u"], dtype=np.float32)
    assert x.shape == (B, C, H, W), x.shape
    assert rand_u.shape == (H, W), rand_u.shape

    u_rep = np.ascontiguousarray(
        np.broadcast_to(rand_u.reshape(1, HW), (P, HW)), dtype=np.float32
    )
    in_maps = []
    for i in range(N_CORES):
        shard = x[i * B_SH : (i + 1) * B_SH].reshape(P, F_TOTAL)
        in_maps.append({"x": shard, "u": u_rep})

    key = BUILDER
    if key not in _NC_CACHE:
        _NC_CACHE[key] = {
            "tile": _build_nc,
            "raw": _build_nc_raw,
            "taper": _build_nc_taper,
            "hw8": lambda: _build_nc_hw(8),
            "hw16": lambda: _build_nc_hw(16),
            "hwtaper": _build_nc_hwtaper,
        }[key]()
    nc = _NC_CACHE[key]

    res = run_bass_kernel_spmd(nc, in_maps, list(range(N_CORES)), trace=trace)
    out = np.empty((B, C, H, W), dtype=np.float32)
    for i in range(N_CORES):
        out[i * B_SH : (i + 1) * B_SH] = res.results[i]["out"].reshape(
            B_SH, C, H, W
        )
    return out, res


def kernel(**inputs: np.ndarray) -> np.ndarray:
    out, _ = _run(inputs, trace=False)
    return out


# revision 33
# speedup vs baseline: 1.0828x; 1.0465x over previous
"""Trainium2 Bass kernel for nn_BatchCropElements: out = x * (rand_u > 0.3).

Full inputs: x [64, 2048, 24, 12] f32, rand_u [24, 12] f32.
Sharding: data-parallel on batch across 8 cores -> per-core x [8, 2048, 24, 12],
viewed flat as [128 partitions, 36864 free] f32 (36864 = 128 spatial planes of
288 = 24*12 elements, so the mask pattern tiles the free dim exactly).
rand_u is replicated to every core (host pre-broadcasts it to [128, 288]).

Per core: build the f32 0/1 mask once in SBUF (threshold + log-doubling
widen to chunk width), then stream 8 chunks of [128, 4608]: HWDGE load on
the SP ring -> DVE tensor-tensor multiply in place -> HWDGE store on the ACT
ring (no GpSimd: avoids its library preamble and SWDGE descriptor costs; the
two rings round-robin packets so reads and writes share the fabric fairly).
Memory-bound: the DMA fabric sustains ~433 GB/s/core combined read+write;
measured best ~100.8 us/core on HW (~9 us fixed NEFF startup + 37.9 MB of
HBM traffic + tail barrier), with device-state noise up to ~120 us.
"""

from contextlib import ExitStack

import numpy as np

import concourse.bass as bass
import concourse.tile as tile
from concourse import bacc, mybir
from concourse.bass_utils import run_bass_kernel_spmd

N_CORES = 8
B, C, H, W = 64, 2048, 24, 12
HW = H * W  # 288
B_SH = B // N_CORES  # 8 batches per core
P = 128
F_TOTAL = B_SH * C * HW // P  # 36864 f32 per partition
F = 4608  # chunk free size (16 spatial planes); F % HW == 0
N_CHUNK = F_TOTAL // F  # 8
PROB = 0.3

_DT = mybir.dt.float32


def _build_nc() -> bass.Bass:
    # Bacc (not raw Bass): its finalize pipeline splits multi-wait sync into
    # event-semaphore chains — TRN2 allows at most 1 wait per instruction.
    nc = bacc.Bacc()
    x = nc.declare_dram_parameter("x", [P, F_TOTAL], _DT, isOutput=False)
    u = nc.declare_dram_parameter("u", [P, HW], _DT, isOutput=False)
    out = nc.declare_dram_parameter("out", [P, F_TOTAL], _DT, isOutput=True)

    # The walrus TensorTensor encoding fits only one embedded sync wait, so
    # the structure keeps every DVE op at <=1 wait: bufs == N_CHUNK (no SBUF
    # slot reuse -> loads need no WAR wait, muls wait only on their own
    # load), in-place multiply, and a 1-element "absorber" copy that soaks up
    # the mask-ready wait so the first mul doesn't carry two.
    with tile.TileContext(nc) as tc:
        with (
            tc.tile_pool(name="upool", bufs=1) as upool,
            tc.tile_pool(name="maskp", bufs=1) as maskp,
            tc.tile_pool(name="scrp", bufs=1) as scrp,
            tc.tile_pool(name="iop", bufs=N_CHUNK // 2) as iop,
        ):
            tu = upool.tile([P, HW], _DT)
            nc.sync.dma_start(out=tu[:], in_=u[:, :])
            bmask = maskp.tile([P, F], _DT)
            nc.vector.tensor_scalar(
                out=bmask[:, 0:HW],
                in0=tu[:],
                scalar1=PROB,
                scalar2=None,
                op0=mybir.AluOpType.is_gt,
            )
            w = HW
            while w < F:
                nc.vector.tensor_copy(out=bmask[:, w : 2 * w], in_=bmask[:, 0:w])
                w *= 2
            scr = scrp.tile([1, 1], _DT)
            nc.vector.tensor_copy(out=scr[:], in_=bmask[0:1, F - 1 : F])

            # Paired chunks: 8 loads on the HWDGE lanes, but one SWDGE store
            # per [P, 2F] double-tile so only 4 DMA-SW lanes are used. Keeps
            # the kernel-tail drain's sem-wait list (1 DVE + 8 HW + 4 SW)
            # under the CTRL struct's capacity, stores at 1 wait (DVE), and
            # stores off the HWDGE lanes where reuse would add a second wait.
            for k in range(N_CHUNK // 2):
                t = iop.tile([P, 2 * F], _DT, name="t")
                for h in range(2):
                    c = 2 * k + h
                    sl = slice(h * F, (h + 1) * F)
                    nc.sync.dma_start(out=t[:, sl], in_=x[:, c * F : (c + 1) * F])
                    nc.vector.tensor_mul(out=t[:, sl], in0=t[:, sl], in1=bmask[:])
                nc.gpsimd.dma_start(
                    out=out[:, 2 * k * F : 2 * (k + 1) * F], in_=t[:]
                )
    nc.finalize()
    return nc


def _build_nc_raw() -> bass.Bass:
    """Raw Bacc variant: hand-rolled semaphores, no Tile tail barrier.

    SP issues the mask load + 8 chunk loads (HWDGE), each with its own
    semaphore (a shared DMA sem cannot order per-chunk completion: the 16
    SDMA engine slots of different in-flight DMAs interleave increments).
    DVE builds the mask then multiplies each chunk in place; GpSimd issues
    one SWDGE store per chunk pair and ends with a single wait for all
    store completions.
    """
    nc = bacc.Bacc()
    x = nc.declare_dram_parameter("x", [P, F_TOTAL], _DT, isOutput=False)
    u = nc.declare_dram_parameter("u", [P, HW], _DT, isOutput=False)
    out = nc.declare_dram_parameter("out", [P, F_TOTAL], _DT, isOutput=True)

    with ExitStack() as ctx:
        tu = ctx.enter_context(nc.sbuf_tensor("tu", [P, HW], _DT))
        bmask = ctx.enter_context(nc.sbuf_tensor("bmask", [P, F], _DT))
        # Double-width buffers: halves are loaded/multiplied per chunk, the
        # whole buffer is stored in one SWDGE DMA per pair.
        pairs = [
            ctx.enter_context(nc.sbuf_tensor(f"pair{k}", [P, 2 * F], _DT))
            for k in range(N_CHUNK // 2)
        ]
        ts = [pairs[c // 2][:, (c % 2) * F : (c % 2 + 1) * F] for c in range(N_CHUNK)]
        ld_sems = [
            ctx.enter_context(nc.semaphore(f"ld{c}")) for c in range(N_CHUNK + 1)
        ]
        mul_sem = ctx.enter_context(nc.semaphore("mul"))
        mk_sem = ctx.enter_context(nc.semaphore("mk"))
        st_sem = ctx.enter_context(nc.semaphore("st"))
        block = ctx.enter_context(nc.Block())

        @block.sync
        def _(sync):
            sync.dma_start(out=tu[:], in_=u[:, :]).then_inc(ld_sems[0], 16)
            for c in range(N_CHUNK):
                sync.dma_start(
                    out=ts[c], in_=x[:, c * F : (c + 1) * F]
                ).then_inc(ld_sems[c + 1], 16)

        @block.vector
        def _(vector):
            vector.wait_ge(ld_sems[0], 16)
            # DVE is pipelined: same-engine RAW chains need explicit sems.
            vector.tensor_scalar(
                out=bmask[:, 0:HW],
                in0=tu[:],
                scalar1=PROB,
                scalar2=None,
                op0=mybir.AluOpType.is_gt,
            ).then_inc(mk_sem, 1)
            w = HW
            n_mk = 1
            while w < F:
                vector.wait_ge(mk_sem, n_mk)
                vector.tensor_copy(
                    out=bmask[:, w : 2 * w], in_=bmask[:, 0:w]
                ).then_inc(mk_sem, 1)
                w *= 2
                n_mk += 1
            for c in range(N_CHUNK):
                if c == 0:
                    vector.wait_ge(mk_sem, n_mk)
                vector.wait_ge(ld_sems[c + 1], 16)
                vector.tensor_tensor(
                    out=ts[c],
                    in0=ts[c],
                    in1=bmask[:],
                    op=mybir.AluOpType.mult,
                ).then_inc(mul_sem, 1)

        @block.gpsimd
        def _(gpsimd):
            for k in range(N_CHUNK // 2):
                gpsimd.wait_ge(mul_sem, 2 * k + 2)
                gpsimd.dma_start(
                    out=out[:, 2 * k * F : 2 * (k + 1) * F],
                    in_=pairs[k][:],
                ).then_inc(st_sem, 16)
            gpsimd.wait_ge(st_sem, 16 * (N_CHUNK // 2))

    nc.finalize()
    return nc


def _build_nc_taper() -> bass.Bass:
    """Tile variant with tapered chunk sizes: big chunks stream first, tiny
    chunks last, so the serial endgame (last load -> last mul -> last store)
    is a few hundred KB instead of 4.5 MB."""
    nc = bacc.Bacc()
    x = nc.declare_dram_parameter("x", [P, F_TOTAL], _DT, isOutput=False)
    u = nc.declare_dram_parameter("u", [P, HW], _DT, isOutput=False)
    out = nc.declare_dram_parameter("out", [P, F_TOTAL], _DT, isOutput=True)

    # chunk sizes in spatial planes (x HW f32 along the free dim); sums to 128
    plan = [16, 16, 16, 16, 16, 16, 16, 8, 4, 2, 1, 1]
    assert sum(plan) == F_TOTAL // HW
    # store groups: consecutive chunk-index pairs share one SBUF buffer
    groups = [(0, 1), (2, 3), (4, 5), (6, 7), (8, 9), (10, 11)]

    with tile.TileContext(nc) as tc:
        with (
            tc.tile_pool(name="upool", bufs=1) as upool,
            tc.tile_pool(name="maskp", bufs=1) as maskp,
            tc.tile_pool(name="iop", bufs=1) as iop,
        ):
            tu = upool.tile([P, HW], _DT)
            nc.sync.dma_start(out=tu[:], in_=u[:, :])
            bmask = maskp.tile([P, F], _DT)
            nc.vector.tensor_scalar(
                out=bmask[:, 0:HW],
                in0=tu[:],
                scalar1=PROB,
                scalar2=None,
                op0=mybir.AluOpType.is_gt,
            )
            w = HW
            while w < F:
                nc.vector.tensor_copy(out=bmask[:, w : 2 * w], in_=bmask[:, 0:w])
                w *= 2

            starts = [sum(plan[:i]) * HW for i in range(len(plan) + 1)]
            for gi, grp in enumerate(groups):
                g_lo, g_hi = starts[grp[0]], starts[grp[-1] + 1]
                t = iop.tile([P, g_hi - g_lo], _DT, name=f"t{gi}")
                for c in grp:
                    lo, hi = starts[c] - g_lo, starts[c + 1] - g_lo
                    nc.sync.dma_start(
                        out=t[:, lo:hi], in_=x[:, starts[c] : starts[c + 1]]
                    )
                    fw = hi - lo
                    nc.vector.tensor_mul(
                        out=t[:, lo:hi], in0=t[:, lo:hi], in1=bmask[:, 0:fw]
                    )
                nc.gpsimd.dma_start(out=out[:, g_lo:g_hi], in_=t[:])
    nc.finalize()
    return nc


def _build_nc_hw(n_chunk: int, swap: bool = False) -> bass.Bass:
    """All-HWDGE variant: loads on the SP ring, stores on the ACT ring, no
    GpSimd at all (drops its library preamble and the ~1us/DMA SWDGE
    descriptor cost). One store per chunk for fine R/W interleave; Bacc's
    finalize legalizes the lane-reuse multi-waits this creates.
    swap=True issues loads from the scalar engine and stores from sync
    (the SP engine spends ~3 us longer in the begin-barrier preamble, so
    issuing loads from ACT may start the read stream earlier)."""
    f = F_TOTAL // n_chunk
    assert f % HW == 0
    nc = bacc.Bacc()
    x = nc.declare_dram_parameter("x", [P, F_TOTAL], _DT, isOutput=False)
    u = nc.declare_dram_parameter("u", [P, HW], _DT, isOutput=False)
    out = nc.declare_dram_parameter("out", [P, F_TOTAL], _DT, isOutput=True)

    ld_eng = nc.scalar if swap else nc.sync
    st_eng = nc.sync if swap else nc.scalar

    with tile.TileContext(nc) as tc:
        with (
            tc.tile_pool(name="upool", bufs=1) as upool,
            tc.tile_pool(name="maskp", bufs=1) as maskp,
            tc.tile_pool(name="iop", bufs=n_chunk) as iop,
        ):
            tu = upool.tile([P, HW], _DT)
            st_eng.dma_start(out=tu[:], in_=u[:, :])
            bmask = maskp.tile([P, f], _DT)
            nc.vector.tensor_scalar(
                out=bmask[:, 0:HW],
                in0=tu[:],
                scalar1=PROB,
                scalar2=None,
                op0=mybir.AluOpType.is_gt,
            )
            w = HW
            while w < f:
                nc.vector.tensor_copy(out=bmask[:, w : 2 * w], in_=bmask[:, 0:w])
                w *= 2
            for c in range(n_chunk):
                t = iop.tile([P, f], _DT, name="t")
                ld_eng.dma_start(out=t[:], in_=x[:, c * f : (c + 1) * f])
                nc.vector.tensor_mul(out=t[:], in0=t[:], in1=bmask[:])
                st_eng.dma_start(out=out[:, c * f : (c + 1) * f], in_=t[:])
    nc.finalize()
    return nc


def _build_nc_hwtaper() -> bass.Bass:
    """hw8's all-HWDGE two-ring structure plus an end-tapered chunk plan so
    the serial endgame (last load -> last mul -> last store) shrinks from
    ~2.25 MB to ~150 KB."""
    plan = [16, 16, 16, 16, 16, 16, 16, 8, 4, 2, 1, 1]
    assert sum(plan) == F_TOTAL // HW
    fmax = max(plan) * HW
    nc = bacc.Bacc()
    x = nc.declare_dram_parameter("x", [P, F_TOTAL], _DT, isOutput=False)
    u = nc.declare_dram_parameter("u", [P, HW], _DT, isOutput=False)
    out = nc.declare_dram_parameter("out", [P, F_TOTAL], _DT, isOutput=True)

    with tile.TileContext(nc) as tc:
        with (
            tc.tile_pool(name="upool", bufs=1) as upool,
            tc.tile_pool(name="maskp", bufs=1) as maskp,
            tc.tile_pool(name="iop", bufs=1) as iop,
        ):
            tu = upool.tile([P, HW], _DT)
            nc.scalar.dma_start(out=tu[:], in_=u[:, :])
            bmask = maskp.tile([P, fmax], _DT)
            nc.vector.tensor_scalar(
                out=bmask[:, 0:HW],
                in0=tu[:],
                scalar1=PROB,
                scalar2=None,
                op0=mybir.AluOpType.is_gt,
            )
            w = HW
            while w < fmax:
                nc.vector.tensor_copy(out=bmask[:, w : 2 * w], in_=bmask[:, 0:w])
                w *= 2
            starts = [sum(plan[:i]) * HW for i in range(len(plan) + 1)]
            for c in range(len(plan)):
                fw = starts[c + 1] - starts[c]
                t = iop.tile([P, fw], _DT, name=f"t{c}")
                nc.sync.dma_start(out=t[:], in_=x[:, starts[c] : starts[c + 1]])
                nc.vector.tensor_mul(out=t[:], in0=t[:], in1=bmask[:, 0:fw])
                nc.scalar.dma_start(out=out[:, starts[c] : starts[c + 1]], in_=t[:])
    nc.finalize()
    return nc


def _build_nc_rawhw() -> bass.Bass:
    """Raw-block variant of hw8: same balanced ring split (loads+mask on the
    SP HWDGE ring, stores on the ACT ring, no GpSimd) but hand-rolled
    semaphores instead of TileContext, trading Tile's drain + double
    all-engine-barrier tail (~8 us) for a single store-completion wait."""
    nc = bacc.Bacc()
    x = nc.declare_dram_parameter("x", [P, F_TOTAL], _DT, isOutput=False)
    u = nc.declare_dram_parameter("u", [P, HW], _DT, isOutput=False)
    out = nc.declare_dram_parameter("out", [P, F_TOTAL], _DT, isOutput=True)

    with ExitStack() as ctx:
        tu = ctx.enter_context(nc.sbuf_tensor("tu", [P, HW], _DT))
        bmask = ctx.enter_context(nc.sbuf_tensor("bmask", [P, F], _DT))
        ts = [
            ctx.enter_context(nc.sbuf_tensor(f"t{c}", [P, F], _DT))
            for c in range(N_CHUNK)
        ]
        ld_sems = [
            ctx.enter_context(nc.semaphore(f"ld{c}")) for c in range(N_CHUNK + 1)
        ]
        mul_sem = ctx.enter_context(nc.semaphore("mul"))
        mk_sem = ctx.enter_context(nc.semaphore("mk"))
        st_sem = ctx.enter_context(nc.semaphore("st"))
        block = ctx.enter_context(nc.Block())

        @block.sync
        def _(sync):
            sync.dma_start(out=tu[:], in_=u[:, :]).then_inc(ld_sems[0], 16)
            for c in range(N_CHUNK):
                sync.dma_start(
                    out=ts[c][:], in_=x[:, c * F : (c + 1) * F]
                ).then_inc(ld_sems[c + 1], 16)

        @block.vector
        def _(vector):
            vector.wait_ge(ld_sems[0], 16)
            # DVE is pipelined: same-engine RAW chains need explicit sems.
            vector.tensor_scalar(
                out=bmask[:, 0:HW],
                in0=tu[:],
                scalar1=PROB,
                scalar2=None,
                op0=mybir.AluOpType.is_gt,
            ).then_inc(mk_sem, 1)
            w = HW
            n_mk = 1
            while w < F:
                vector.wait_ge(mk_sem, n_mk)
                vector.tensor_copy(
                    out=bmask[:, w : 2 * w], in_=bmask[:, 0:w]
                ).then_inc(mk_sem, 1)
                w *= 2
                n_mk += 1
            for c in range(N_CHUNK):
                if c == 0:
                    vector.wait_ge(mk_sem, n_mk)
                vector.wait_ge(ld_sems[c + 1], 16)
                vector.tensor_tensor(
                    out=ts[c][:],
                    in0=ts[c][:],
                    in1=bmask[:],
                    op=mybir.AluOpType.mult,
                ).then_inc(mul_sem, 1)

        @block.scalar
        def _(scalar):
            for c in range(N_CHUNK):
                scalar.wait_ge(mul_sem, c + 1)
                scalar.dma_start(
                    out=out[:, c * F : (c + 1) * F], in_=ts[c][:]
                ).then_inc(st_sem, 16)
            scalar.wait_ge(st_sem, 16 * N_CHUNK)

    nc.finalize()
    return nc


BUILDER = "hw8"
_NC_CACHE: dict = {}


def _run(inputs: dict, trace: bool = False):
    x = np.ascontiguousarray(inputs["x"], dtype=np.float32)
    rand_u = np.ascontiguousarray(inputs["rand_u"], dtype=np.float32)
    assert x.shape == (B, C, H, W), x.shape
    assert rand_u.shape == (H, W), rand_u.shape

    u_rep = np.ascontiguousarray(
        np.broadcast_to(rand_u.reshape(1, HW), (P, HW)), dtype=np.float32
    )
    in_maps = []
    for i in range(N_CORES):
        shard = x[i * B_SH : (i + 1) * B_SH].reshape(P, F_TOTAL)
        in_maps.append({"x": shard, "u": u_rep})

    key = BUILDER
    if key not in _NC_CACHE:
        _NC_CACHE[key] = {
            "tile": _build_nc,
            "raw": _build_nc_raw,
            "taper": _build_nc_taper,
            "hw8": lambda: _build_nc_hw(8),
            "hw8s": lambda: _build_nc_hw(8, swap=True),
            "hw16": lambda: _build_nc_hw(16),
            "hwtaper": _build_nc_hwtaper,
            "rawhw": _build_nc_rawhw,
        }[key]()
    nc = _NC_CACHE[key]

    res = run_bass_kernel_spmd(nc, in_maps, list(range(N_CORES)), trace=trace)
    out = np.empty((B, C, H, W), dtype=np.float32)
    for i in range(N_CORES):
        out[i * B_SH : (i + 1) * B_SH] = res.results[i]["out"].reshape(
            B_SH, C, H, W
        )
    return out, res


def kernel(**inputs: np.ndarray) -> np.ndarray:
    out, _ = _run(inputs, trace=False)
    return out
